# revision 2
# baseline (speedup 1.0000x reference)
"""Paged-attention decode kernel v2 for 8 TRN2 NeuronCores (SPMD, data-parallel).

Problem: nn_Attention_15659450761267 (sparse_attention).
  S=64 seqs, H=32 query heads, HKV=8 kv heads (GQA g=4), D=128, BS=16,
  MAX_BLOCKS=128, T=2048, f32 caches [8192,16,8,128].

v2 vs v1: the host repacks each core's K cache TRANSPOSED
([HKV, D, T] per sequence) while it is already gathering per-core
slabs, so the device does ZERO K transposes and ZERO K^T PSUM->SBUF
copies (v1 spent ~42us of PE and ~77us of DVE per rep on those).
q^T*SCALE and the new-token k^T are also prepared host-side. The
device pipeline per 128-position chunk is just:
  ST[t,(h,g)] = KT_h_chunk^T @ qt_h     (8 matmuls, LDW-dominated)
  p~ = exp(ST + mask_col)               (one ACT op, PSUM->SBUF f32r)
  PV += p~^T @ V ; sums += p~^T @ ones  (3 matmuls, PSUM-accumulated)
With PSUM banks freed, pv/sums go double-buffered so consecutive
sequences overlap through the epilogue. All engines sit well below the
~190us/core HBM roofline; the kernel is memory-bound end to end.

Host-side layout per core c (sequences assigned by snake deal):
  qh  [S_LOC, D, H]   = q^T * SCALE (f32, read as f32r)
  knh [S_LOC, D, HKV] = new-token k^T
  vn  [S_LOC, HKV*D]
  kct [S_LOC, HKV, D, T] = K cache slab transposed (only needed chunks)
  vc  [S_LOC, T, HKV*D]  = V cache slab (position-major, as v1)
"""

import numpy as np

S = 64
H = 32
HKV = 8
G = H // HKV  # 4
D = 128
BS = 16
MAX_BLOCKS = 128
T = MAX_BLOCKS * BS  # 2048
SCALE = 0.08838834764831845
NCORES = 8
S_LOC = S // NCORES  # 8
NEG = -1.0e30
CHUNK = 128          # positions per chunk (one ST tile / mask column)
NCHUNK = T // CHUNK  # 16
BLK = 512            # positions per K/V-load block
NBLK = T // BLK      # 4
CPB = BLK // CHUNK   # 4

_nc_cache = {}


def _build_nc(chunk_counts=(NCHUNK,) * S_LOC, reps=1, kv_bufs=3):
    import concourse.mybir as mybir
    import concourse.tile as tile
    from concourse import bacc

    f32 = mybir.dt.float32
    f32r = mybir.dt.float32r
    i32 = mybir.dt.int32
    Alu = mybir.AluOpType
    Act = mybir.ActivationFunctionType

    nc = bacc.Bacc("TRN2", target_bir_lowering=False, debug=False,
                   num_devices=NCORES)
    qh_d = nc.dram_tensor("qh", [S_LOC, D, H], f32r, kind="ExternalInput")
    knh_d = nc.dram_tensor("knh", [S_LOC, D, HKV], f32r, kind="ExternalInput")
    vn_d = nc.dram_tensor("vn", [S_LOC, HKV * D], f32r, kind="ExternalInput")
    kct_d = nc.dram_tensor("kct", [S_LOC, HKV, D, T], f32r,
                           kind="ExternalInput")
    vc_d = nc.dram_tensor("vc", [S_LOC, T, HKV * D], f32r,
                          kind="ExternalInput")
    cl_d = nc.dram_tensor("cl", [1, S_LOC], i32, kind="ExternalInput")
    out_d = nc.dram_tensor("out", [S_LOC, H, D], f32, kind="ExternalOutput")

    with tile.TileContext(nc) as tc:
        with (
            tc.tile_pool(name="const", bufs=1) as constp,
            tc.tile_pool(name="kt", bufs=kv_bufs) as kpool,
            tc.tile_pool(name="vchunk", bufs=kv_bufs) as vpool,
            tc.tile_pool(name="stexp", bufs=6) as stpool,
            tc.tile_pool(name="small", bufs=2) as smpool,
            tc.tile_pool(name="ps_st", bufs=2, space="PSUM") as ps_st,
            tc.tile_pool(name="ps_pv", bufs=2, space="PSUM") as ps_pv,
            tc.tile_pool(name="ps_sums", bufs=1, space="PSUM") as ps_sums,
            tc.tile_pool(name="ps_small", bufs=1, space="PSUM") as ps_small,
        ):
            onesf = constp.tile([128, G], f32)
            nc.vector.memset(onesf[:], 1.0)
            ones_r = constp.tile([128, G], f32r)
            nc.vector.tensor_copy(ones_r[:], onesf[:])

            # posCols[p, j] = j*128 + p  (position of partition p in chunk j)
            posc_i = constp.tile([CHUNK, NCHUNK], i32)
            nc.gpsimd.iota(posc_i[:], pattern=[[CHUNK, NCHUNK]], base=0,
                           channel_multiplier=1)
            posc = constp.tile([CHUNK, NCHUNK], f32)
            nc.vector.tensor_copy(posc[:], posc_i[:])

            # context_lens -> f32 (cl - 1), broadcast over 128 partitions
            cli = constp.tile([1, S_LOC], i32)
            nc.sync.dma_start(cli[:], cl_d[:])
            clf = constp.tile([1, S_LOC], f32)
            nc.vector.tensor_copy(clf[:], cli[:])
            nc.vector.tensor_scalar_add(clf[:], clf[:], -1.0)
            clb = constp.tile([CHUNK, S_LOC], f32)
            nc.gpsimd.partition_broadcast(clb[:], clf[:])

            for s in [ss for _ in range(reps) for ss in range(S_LOC)]:
                # ---- prologue: q^T (pre-scaled), new-token k^T / v ----
                qt_sb = smpool.tile([D, H], f32r, tag="qt")
                nc.sync.dma_start(qt_sb[:], qh_d[s])
                ktn_sb = smpool.tile([D, HKV], f32r, tag="ktn")
                nc.sync.dma_start(ktn_sb[:], knh_d[s])
                vn_sb = smpool.tile([1, HKV * D], f32r, tag="vn")
                nc.sync.dma_start(vn_sb[:], vn_d[s][None, :])

                pv_ps = ps_pv.tile([H, HKV * D], f32, tag="pv")
                sums_ps = ps_sums.tile([H, G], f32, tag="sums")

                # ---- new token: p~_new row, appended as K=1 matmuls ----
                ptn_ps = ps_small.tile([1, H], f32, tag="misc")
                for h in range(HKV):
                    nc.tensor.matmul(ptn_ps[:, G * h:G * (h + 1)],
                                     ktn_sb[:, h:h + 1],
                                     qt_sb[:, G * h:G * (h + 1)],
                                     start=True, stop=True)
                ptn_sb = smpool.tile([1, H], f32r, tag="ptn_sb")
                nc.scalar.activation(ptn_sb[:], ptn_ps[:], Act.Exp)
                nch = chunk_counts[s]
                ntstop = (nch == 0)
                nc.tensor.matmul(pv_ps[:, :512], ptn_sb[:], vn_sb[:, :512],
                                 start=True, stop=ntstop)
                nc.tensor.matmul(pv_ps[:, 512:], ptn_sb[:], vn_sb[:, 512:],
                                 start=True, stop=ntstop)
                nc.tensor.matmul(sums_ps[:], ptn_sb[:], ones_r[:1, :],
                                 start=True, stop=ntstop)

                kct_v = kct_d[s].rearrange("h d p -> d h p")
                vc_v = vc_d[s].rearrange("(c p) d -> p c d", p=CHUNK)

                nblocks = (nch + CPB - 1) // CPB
                for b in range(nblocks):
                    cpb = min(CPB, nch - b * CPB)
                    np_ = cpb * CHUNK
                    kt_sb = kpool.tile([D, HKV, BLK], f32r, tag="kt")
                    nc.sync.dma_start(
                        kt_sb[:, :, :np_],
                        kct_v[:, :, b * BLK:b * BLK + np_])
                    v_sb = vpool.tile([CHUNK, CPB, HKV * D], f32r,
                                      tag="vchunk")
                    nc.sync.dma_start(
                        v_sb[:, :cpb], vc_v[:, b * CPB:b * CPB + cpb, :])

                    for c2 in range(cpb):
                        c = b * CPB + c2
                        # ST[t, (h,g)] = k_t . q_(h,g) (pre-scaled)
                        st_ps = ps_st.tile([CHUNK, H], f32, tag="st")
                        for h in range(HKV):
                            nc.tensor.matmul(
                                st_ps[:, G * h:G * (h + 1)],
                                kt_sb[:, h, c2 * CHUNK:(c2 + 1) * CHUNK],
                                qt_sb[:, G * h:G * (h + 1)],
                                start=True, stop=True)
                        # mask column: -1e30 where position >= cl-1
                        mc = smpool.tile([CHUNK, 1], f32, tag="mc")
                        nc.vector.tensor_scalar(
                            mc[:], posc[:, c:c + 1], clb[:, s:s + 1], NEG,
                            op0=Alu.is_ge, op1=Alu.mult)
                        # p~ = exp(ST + mask): one ACT op, PSUM -> SBUF f32r
                        st_exp = stpool.tile([CHUNK, H], f32r, tag="stexp")
                        nc.scalar.activation(st_exp[:], st_ps[:], Act.Exp,
                                             bias=mc[:, 0:1])
                        # PV and denominator accumulation
                        last = (c == nch - 1)
                        nc.tensor.matmul(pv_ps[:, :512], st_exp[:],
                                         v_sb[:, c2, :512],
                                         start=False, stop=last)
                        nc.tensor.matmul(pv_ps[:, 512:], st_exp[:],
                                         v_sb[:, c2, 512:],
                                         start=False, stop=last)
                        nc.tensor.matmul(sums_ps[:], st_exp[:], ones_r[:],
                                         start=False, stop=last)

                # ---- epilogue: out = PV / sums, band-DMA to DRAM ----
                sums_sb = smpool.tile([H, 1], f32, tag="sums_sb")
                nc.vector.tensor_copy(sums_sb[:], sums_ps[:, 0:1])
                rcp = smpool.tile([H, 1], f32, tag="rcp")
                nc.vector.reciprocal(rcp[:], sums_sb[:])
                pv_stage = smpool.tile([H, HKV * D], f32, tag="pvstage")
                nc.vector.tensor_scalar(pv_stage[:], pv_ps[:], rcp[:, 0:1],
                                        None, op0=Alu.mult)
                for h in range(HKV):
                    nc.scalar.dma_start(
                        out_d[s, G * h:G * (h + 1), :],
                        pv_stage[G * h:G * (h + 1), h * D:(h + 1) * D])

    nc.compile()
    return nc


def _get_nc(chunk_counts):
    key = tuple(chunk_counts)
    if key not in _nc_cache:
        _nc_cache[key] = _build_nc(chunk_counts=key)
    return _nc_cache[key]


def _plan(q, k, v, k_cache, v_cache, block_tables, context_lens,
          slot_mapping):
    """Sort sequences by context length, snake-deal to (core, slot), and
    compute per-slot static chunk counts (max over cores in each slot)."""
    q = np.ascontiguousarray(np.asarray(q, np.float32))
    k = np.ascontiguousarray(np.asarray(k, np.float32))
    v = np.ascontiguousarray(np.asarray(v, np.float32))
    kc = np.asarray(k_cache, np.float32)
    vc = np.asarray(v_cache, np.float32)
    bt = np.asarray(block_tables)
    cl = np.asarray(context_lens, np.int32)

    expect = np.arange(S * MAX_BLOCKS, dtype=np.int64).reshape(S, MAX_BLOCKS)
    if not np.array_equal(np.asarray(bt, np.int64), expect):
        # General fallback (never hit for the spec's arange tables): gather
        # each sequence's blocks into contiguous order on the host.
        kc4 = kc[np.asarray(bt, np.int64)].reshape(S, T, HKV, D)
        vc2 = vc[np.asarray(bt, np.int64)].reshape(S, T, HKV * D)
    else:
        kc4 = kc.reshape(S, T, HKV, D)
        vc2 = vc.reshape(S, T, HKV * D)

    # cached chunks needed for positions 0 .. cl-2
    need = np.ceil(np.maximum(cl - 1, 0) / CHUNK).astype(np.int64)
    order = np.argsort(-need, kind="stable")  # desc by need
    # snake deal: rank group j -> slot j; within group alternate direction
    assign = np.empty((NCORES, S_LOC), np.int64)
    for j in range(S_LOC):
        grp = order[j * NCORES:(j + 1) * NCORES]
        if j % 2 == 1:
            grp = grp[::-1]
        assign[:, j] = grp
    chunk_counts = tuple(int(need[assign[:, j]].max()) for j in range(S_LOC))

    in_maps = []
    for c in range(NCORES):
        idx = assign[c]
        # K transposed per sequence: [s, HKV, D, T]; only transpose the
        # chunks that will actually be read (chunk_counts is slot-wise).
        kct = np.zeros((S_LOC, HKV, D, T), np.float32)
        for j in range(S_LOC):
            npos = chunk_counts[j] * CHUNK
            if npos:
                kct[j, :, :, :npos] = kc4[idx[j], :npos].transpose(1, 2, 0)
        in_maps.append({
            "qh": np.ascontiguousarray(
                q[idx].transpose(0, 2, 1) * np.float32(SCALE)),
            "knh": np.ascontiguousarray(k[idx].transpose(0, 2, 1)),
            "vn": np.ascontiguousarray(v[idx].reshape(S_LOC, HKV * D)),
            "kct": kct,
            "vc": np.ascontiguousarray(vc2[idx]),
            "cl": np.ascontiguousarray(cl[idx]).reshape(1, S_LOC),
        })
    return in_maps, assign, chunk_counts


def _prep_shards(q, k, v, k_cache, v_cache, block_tables, context_lens,
                 slot_mapping):
    in_maps, _, _ = _plan(q, k, v, k_cache, v_cache, block_tables,
                          context_lens, slot_mapping)
    return in_maps


def kernel(q, k, v, k_cache, v_cache, block_tables, context_lens,
           slot_mapping) -> np.ndarray:
    from concourse.bass_utils import run_bass_kernel_spmd

    in_maps, assign, chunk_counts = _plan(
        q, k, v, k_cache, v_cache, block_tables, context_lens, slot_mapping)
    nc = _get_nc(chunk_counts)
    res = run_bass_kernel_spmd(nc, in_maps, core_ids=list(range(NCORES)),
                               trace=False)
    out = np.empty((S, H, D), np.float32)
    for c in range(NCORES):
        out[assign[c]] = res.results[c]["out"]
    return np.ascontiguousarray(out)


# revision 3
# speedup vs baseline: 1.4613x; 1.4613x over previous
"""Paged-attention decode kernel v3 (bf16) for 8 TRN2 NeuronCores (SPMD, data-parallel).

Problem: nn_Attention_15659450761267 (sparse_attention).
  S=64 seqs, H=32 query heads, HKV=8 kv heads (GQA g=4), D=128, BS=16,
  MAX_BLOCKS=128, T=2048, f32 caches [8192,16,8,128].

v3 = v2 + bf16: since the host repacks all per-core slabs anyway, it
downcasts K, V, q to bf16 during packing, HALVING device HBM traffic
(the accuracy budget is 2e-2; bf16 costs ~5e-3). PE matmuls run
bf16 x bf16 -> f32 PSUM (same 1 elem/cycle, plus FWL fast weight
loads); exp outputs bf16 for the PV weights. Epilogue stays f32.

v2 vs v1: the host repacks each core's K cache TRANSPOSED
([HKV, D, T] per sequence) while it is already gathering per-core
slabs, so the device does ZERO K transposes and ZERO K^T PSUM->SBUF
copies (v1 spent ~42us of PE and ~77us of DVE per rep on those).
q^T*SCALE and the new-token k^T are also prepared host-side. The
device pipeline per 128-position chunk is just:
  ST[t,(h,g)] = KT_h_chunk^T @ qt_h     (8 matmuls, LDW-dominated)
  p~ = exp(ST + mask_col)               (one ACT op, PSUM->SBUF f32r)
  PV += p~^T @ V ; sums += p~^T @ ones  (3 matmuls, PSUM-accumulated)
With PSUM banks freed, pv/sums go double-buffered so consecutive
sequences overlap through the epilogue. All engines sit well below the
~190us/core HBM roofline; the kernel is memory-bound end to end.

Host-side layout per core c (sequences assigned by snake deal):
  qh  [S_LOC, D, H]   = q^T * SCALE (f32, read as f32r)
  knh [S_LOC, D, HKV] = new-token k^T
  vn  [S_LOC, HKV*D]
  kct [S_LOC, HKV, D, T] = K cache slab transposed (only needed chunks)
  vc  [S_LOC, T, HKV*D]  = V cache slab (position-major, as v1)
"""

import numpy as np

try:
    from ml_dtypes import bfloat16 as np_bf16
except ImportError:  # pragma: no cover
    import jax.numpy as _jnp
    np_bf16 = _jnp.bfloat16

S = 64
H = 32
HKV = 8
G = H // HKV  # 4
D = 128
BS = 16
MAX_BLOCKS = 128
T = MAX_BLOCKS * BS  # 2048
SCALE = 0.08838834764831845
NCORES = 8
S_LOC = S // NCORES  # 8
NEG = -1.0e30
CHUNK = 128          # positions per chunk (one ST tile / mask column)
NCHUNK = T // CHUNK  # 16
BLK = 512            # positions per K/V-load block
NBLK = T // BLK      # 4
CPB = BLK // CHUNK   # 4

_nc_cache = {}


def _build_nc(chunk_counts=(NCHUNK,) * S_LOC, reps=1, kv_bufs=3):
    import concourse.mybir as mybir
    import concourse.tile as tile
    from concourse import bacc

    f32 = mybir.dt.float32
    f32r = mybir.dt.float32r
    bf16 = mybir.dt.bfloat16
    i32 = mybir.dt.int32
    Alu = mybir.AluOpType
    Act = mybir.ActivationFunctionType

    nc = bacc.Bacc("TRN2", target_bir_lowering=False, debug=False,
                   num_devices=NCORES)
    qh_d = nc.dram_tensor("qh", [S_LOC, D, H], bf16, kind="ExternalInput")
    knh_d = nc.dram_tensor("knh", [S_LOC, D, HKV], bf16, kind="ExternalInput")
    vn_d = nc.dram_tensor("vn", [S_LOC, HKV * D], bf16, kind="ExternalInput")
    kct_d = nc.dram_tensor("kct", [S_LOC, HKV, D, T], bf16,
                           kind="ExternalInput")
    vc_d = nc.dram_tensor("vc", [S_LOC, T, HKV * D], bf16,
                          kind="ExternalInput")
    cl_d = nc.dram_tensor("cl", [1, S_LOC], i32, kind="ExternalInput")
    out_d = nc.dram_tensor("out", [S_LOC, H, D], f32, kind="ExternalOutput")

    with tile.TileContext(nc) as tc:
        with (
            tc.tile_pool(name="const", bufs=1) as constp,
            tc.tile_pool(name="kt", bufs=kv_bufs) as kpool,
            tc.tile_pool(name="vchunk", bufs=kv_bufs) as vpool,
            tc.tile_pool(name="stexp", bufs=6) as stpool,
            tc.tile_pool(name="small", bufs=2) as smpool,
            tc.tile_pool(name="ps_st", bufs=2, space="PSUM") as ps_st,
            tc.tile_pool(name="ps_pv", bufs=2, space="PSUM") as ps_pv,
            tc.tile_pool(name="ps_sums", bufs=1, space="PSUM") as ps_sums,
            tc.tile_pool(name="ps_small", bufs=1, space="PSUM") as ps_small,
        ):
            onesf = constp.tile([128, G], f32)
            nc.vector.memset(onesf[:], 1.0)
            ones_r = constp.tile([128, G], bf16)
            nc.vector.tensor_copy(ones_r[:], onesf[:])

            # posCols[p, j] = j*128 + p  (position of partition p in chunk j)
            posc_i = constp.tile([CHUNK, NCHUNK], i32)
            nc.gpsimd.iota(posc_i[:], pattern=[[CHUNK, NCHUNK]], base=0,
                           channel_multiplier=1)
            posc = constp.tile([CHUNK, NCHUNK], f32)
            nc.vector.tensor_copy(posc[:], posc_i[:])

            # context_lens -> f32 (cl - 1), broadcast over 128 partitions
            cli = constp.tile([1, S_LOC], i32)
            nc.sync.dma_start(cli[:], cl_d[:])
            clf = constp.tile([1, S_LOC], f32)
            nc.vector.tensor_copy(clf[:], cli[:])
            nc.vector.tensor_scalar_add(clf[:], clf[:], -1.0)
            clb = constp.tile([CHUNK, S_LOC], f32)
            nc.gpsimd.partition_broadcast(clb[:], clf[:])

            for s in [ss for _ in range(reps) for ss in range(S_LOC)]:
                # ---- prologue: q^T (pre-scaled), new-token k^T / v ----
                qt_sb = smpool.tile([D, H], bf16, tag="qt")
                nc.sync.dma_start(qt_sb[:], qh_d[s])
                ktn_sb = smpool.tile([D, HKV], bf16, tag="ktn")
                nc.sync.dma_start(ktn_sb[:], knh_d[s])
                vn_sb = smpool.tile([1, HKV * D], bf16, tag="vn")
                nc.sync.dma_start(vn_sb[:], vn_d[s][None, :])

                pv_ps = ps_pv.tile([H, HKV * D], f32, tag="pv")
                sums_ps = ps_sums.tile([H, G], f32, tag="sums")

                # ---- new token: p~_new row, appended as K=1 matmuls ----
                ptn_ps = ps_small.tile([1, H], f32, tag="misc")
                for h in range(HKV):
                    nc.tensor.matmul(ptn_ps[:, G * h:G * (h + 1)],
                                     ktn_sb[:, h:h + 1],
                                     qt_sb[:, G * h:G * (h + 1)],
                                     start=True, stop=True)
                ptn_sb = smpool.tile([1, H], bf16, tag="ptn_sb")
                nc.scalar.activation(ptn_sb[:], ptn_ps[:], Act.Exp)
                nch = chunk_counts[s]
                ntstop = (nch == 0)
                nc.tensor.matmul(pv_ps[:, :512], ptn_sb[:], vn_sb[:, :512],
                                 start=True, stop=ntstop)
                nc.tensor.matmul(pv_ps[:, 512:], ptn_sb[:], vn_sb[:, 512:],
                                 start=True, stop=ntstop)
                nc.tensor.matmul(sums_ps[:], ptn_sb[:], ones_r[:1, :],
                                 start=True, stop=ntstop)

                kct_v = kct_d[s].rearrange("h d p -> d h p")
                vc_v = vc_d[s].rearrange("(c p) d -> p c d", p=CHUNK)

                nblocks = (nch + CPB - 1) // CPB
                for b in range(nblocks):
                    cpb = min(CPB, nch - b * CPB)
                    np_ = cpb * CHUNK
                    kt_sb = kpool.tile([D, HKV, BLK], bf16, tag="kt")
                    nc.sync.dma_start(
                        kt_sb[:, :, :np_],
                        kct_v[:, :, b * BLK:b * BLK + np_])
                    v_sb = vpool.tile([CHUNK, CPB, HKV * D], bf16,
                                      tag="vchunk")
                    nc.sync.dma_start(
                        v_sb[:, :cpb], vc_v[:, b * CPB:b * CPB + cpb, :])

                    for c2 in range(cpb):
                        c = b * CPB + c2
                        # ST[t, (h,g)] = k_t . q_(h,g) (pre-scaled)
                        st_ps = ps_st.tile([CHUNK, H], f32, tag="st")
                        for h in range(HKV):
                            nc.tensor.matmul(
                                st_ps[:, G * h:G * (h + 1)],
                                kt_sb[:, h, c2 * CHUNK:(c2 + 1) * CHUNK],
                                qt_sb[:, G * h:G * (h + 1)],
                                start=True, stop=True)
                        # mask column: -1e30 where position >= cl-1
                        mc = smpool.tile([CHUNK, 1], f32, tag="mc")
                        nc.vector.tensor_scalar(
                            mc[:], posc[:, c:c + 1], clb[:, s:s + 1], NEG,
                            op0=Alu.is_ge, op1=Alu.mult)
                        # p~ = exp(ST + mask): one ACT op, PSUM -> SBUF f32r
                        st_exp = stpool.tile([CHUNK, H], bf16, tag="stexp")
                        nc.scalar.activation(st_exp[:], st_ps[:], Act.Exp,
                                             bias=mc[:, 0:1])
                        # PV and denominator accumulation
                        last = (c == nch - 1)
                        nc.tensor.matmul(pv_ps[:, :512], st_exp[:],
                                         v_sb[:, c2, :512],
                                         start=False, stop=last)
                        nc.tensor.matmul(pv_ps[:, 512:], st_exp[:],
                                         v_sb[:, c2, 512:],
                                         start=False, stop=last)
                        nc.tensor.matmul(sums_ps[:], st_exp[:], ones_r[:],
                                         start=False, stop=last)

                # ---- epilogue: out = PV / sums, band-DMA to DRAM ----
                sums_sb = smpool.tile([H, 1], f32, tag="sums_sb")
                nc.vector.tensor_copy(sums_sb[:], sums_ps[:, 0:1])
                rcp = smpool.tile([H, 1], f32, tag="rcp")
                nc.vector.reciprocal(rcp[:], sums_sb[:])
                pv_stage = smpool.tile([H, HKV * D], f32, tag="pvstage")
                nc.vector.tensor_scalar(pv_stage[:], pv_ps[:], rcp[:, 0:1],
                                        None, op0=Alu.mult)
                for h in range(HKV):
                    nc.scalar.dma_start(
                        out_d[s, G * h:G * (h + 1), :],
                        pv_stage[G * h:G * (h + 1), h * D:(h + 1) * D])

    nc.compile()
    return nc


def _get_nc(chunk_counts):
    key = tuple(chunk_counts)
    if key not in _nc_cache:
        _nc_cache[key] = _build_nc(chunk_counts=key)
    return _nc_cache[key]


def _plan(q, k, v, k_cache, v_cache, block_tables, context_lens,
          slot_mapping):
    """Sort sequences by context length, snake-deal to (core, slot), and
    compute per-slot static chunk counts (max over cores in each slot)."""
    q = np.ascontiguousarray(np.asarray(q, np.float32))
    k = np.ascontiguousarray(np.asarray(k, np.float32))
    v = np.ascontiguousarray(np.asarray(v, np.float32))
    kc = np.asarray(k_cache, np.float32)
    vc = np.asarray(v_cache, np.float32)
    bt = np.asarray(block_tables)
    cl = np.asarray(context_lens, np.int32)

    expect = np.arange(S * MAX_BLOCKS, dtype=np.int64).reshape(S, MAX_BLOCKS)
    if not np.array_equal(np.asarray(bt, np.int64), expect):
        # General fallback (never hit for the spec's arange tables): gather
        # each sequence's blocks into contiguous order on the host.
        kc4 = kc[np.asarray(bt, np.int64)].reshape(S, T, HKV, D)
        vc2 = vc[np.asarray(bt, np.int64)].reshape(S, T, HKV * D)
    else:
        kc4 = kc.reshape(S, T, HKV, D)
        vc2 = vc.reshape(S, T, HKV * D)

    # cached chunks needed for positions 0 .. cl-2
    need = np.ceil(np.maximum(cl - 1, 0) / CHUNK).astype(np.int64)
    order = np.argsort(-need, kind="stable")  # desc by need
    # snake deal: rank group j -> slot j; within group alternate direction
    assign = np.empty((NCORES, S_LOC), np.int64)
    for j in range(S_LOC):
        grp = order[j * NCORES:(j + 1) * NCORES]
        if j % 2 == 1:
            grp = grp[::-1]
        assign[:, j] = grp
    chunk_counts = tuple(int(need[assign[:, j]].max()) for j in range(S_LOC))

    in_maps = []
    for c in range(NCORES):
        idx = assign[c]
        # K transposed per sequence: [s, HKV, D, T]; only transpose the
        # chunks that will actually be read (chunk_counts is slot-wise).
        kct = np.zeros((S_LOC, HKV, D, T), np_bf16)
        for j in range(S_LOC):
            npos = chunk_counts[j] * CHUNK
            if npos:
                kct[j, :, :, :npos] = (
                    kc4[idx[j], :npos].transpose(1, 2, 0).astype(np_bf16))
        in_maps.append({
            "qh": np.ascontiguousarray(
                (q[idx].transpose(0, 2, 1) * np.float32(SCALE))
                .astype(np_bf16)),
            "knh": np.ascontiguousarray(
                k[idx].transpose(0, 2, 1).astype(np_bf16)),
            "vn": np.ascontiguousarray(
                v[idx].reshape(S_LOC, HKV * D).astype(np_bf16)),
            "kct": kct,
            "vc": np.ascontiguousarray(vc2[idx].astype(np_bf16)),
            "cl": np.ascontiguousarray(cl[idx]).reshape(1, S_LOC),
        })
    return in_maps, assign, chunk_counts


def _prep_shards(q, k, v, k_cache, v_cache, block_tables, context_lens,
                 slot_mapping):
    in_maps, _, _ = _plan(q, k, v, k_cache, v_cache, block_tables,
                          context_lens, slot_mapping)
    return in_maps


def kernel(q, k, v, k_cache, v_cache, block_tables, context_lens,
           slot_mapping) -> np.ndarray:
    from concourse.bass_utils import run_bass_kernel_spmd

    in_maps, assign, chunk_counts = _plan(
        q, k, v, k_cache, v_cache, block_tables, context_lens, slot_mapping)
    nc = _get_nc(chunk_counts)
    res = run_bass_kernel_spmd(nc, in_maps, core_ids=list(range(NCORES)),
                               trace=False)
    out = np.empty((S, H, D), np.float32)
    for c in range(NCORES):
        out[assign[c]] = res.results[c]["out"]
    return np.ascontiguousarray(out)
